# revision 4
# baseline (speedup 1.0000x reference)
"""Trainium2 Bass kernel for nn_NewCNNEncoder (dense CNN encoder over one-hot boards).

Strategy (pure data parallel over 8 NeuronCores, 8192 samples each):
  - The input x [B, 25] (values 0..16) is one-hot encoded ON CHIP via
    broadcast-matmul + is_equal compare, in three layouts matched to the
    three depthwise-conv branches (full / horizontal / vertical).
  - All convolutions are expressed as dense matmuls with activations kept
    in [features-on-partitions, batch-free] layout; the final conv_out
    layer flips to [batch-on-partitions, features-free] so the output DMA
    is contiguous.
  - Matmuls run in float32r (full-rate fp32 mode) except the first-layer
    broadcast and the last layer, which run in bf16.
  - leaky_relu(+bias) epilogues are single ScalarE activation ops reading
    PSUM directly.
"""

import sys

sys.path.insert(0, "/opt/trn_rl_repo")

import numpy as np
import ml_dtypes

import concourse.mybir as mybir
import concourse.tile as tile
from concourse import bacc
from concourse.bass_utils import run_bass_kernel_spmd

NCORES = 8
B_FULL = 65536
BC = B_FULL // NCORES  # 8192 per core
NT = 512               # batch tile (samples per pipeline tile)
NTILES = BC // NT      # 16

NC_ = 25   # cells (5x5 board)
NCL = 17   # classes
MULT = 16
OC = 64
OF = 1600
SLOPE = 0.01

F32 = mybir.dt.float32
F32R = mybir.dt.float32r
BF16 = mybir.dt.bfloat16
BF16NP = ml_dtypes.bfloat16
LRELU = mybir.ActivationFunctionType.Lrelu
EQ = mybir.AluOpType.is_equal

# one-hot row chunking over 425 rows (p = 25c + l)
FULL_CH = [(0, 128), (128, 128), (256, 128), (384, 41)]
# per-r (and per-j) L1 output col chunks over 272 (16c + m)
H_CH = [(0, 128), (128, 128), (256, 16)]
# L2-full output chunks over 320
F2_CH = [(0, 128), (128, 128), (256, 64)]
# act2 (cat) K-chunk sizes: 7x128 + 65 (last = vert j4 (64) + ones row)
A2_SIZES = [128] * 7 + [65]
# where each L2 output block lands in the A2 tiles: branch -> (tile, part_off)
H_DST = {0: (2, 64), 1: (3, 0), 2: (3, 64), 3: (4, 0), 4: (4, 64)}
V_DST = {0: (5, 0), 1: (5, 64), 2: (6, 0), 3: (6, 64), 4: (7, 0)}


def _full_blocks():
    """For each full-branch M-chunk, the list of one-hot K-chunks with a
    nonzero block of the block-diagonal depthwise matrix."""
    out = []
    for m0, mp in FULL_CH:
        c_lo, c_hi = m0 // 25, (m0 + mp - 1) // 25
        r_lo, r_hi = 25 * c_lo, 25 * c_hi + 25
        kcs = [kc for kc, (k0, kp) in enumerate(FULL_CH)
               if k0 < r_hi and k0 + kp > r_lo]
        out.append(kcs)
    return out


FULL_BLOCKS = _full_blocks()


def _build_nc():
    nc = bacc.Bacc("TRN2", target_bir_lowering=False, debug=False)

    # ---- DRAM I/O ----
    d_xt = nc.dram_tensor("xt", [NC_, BC], BF16, kind="ExternalInput")
    d_sf = nc.dram_tensor("sf", [NC_, 425], BF16, kind="ExternalInput")
    d_sh = nc.dram_tensor("sh", [NC_, 425], BF16, kind="ExternalInput")
    d_sv = nc.dram_tensor("sv", [NC_, 425], BF16, kind="ExternalInput")
    d_clsf = nc.dram_tensor("clsf", [128, 4], F32, kind="ExternalInput")
    d_clsh = nc.dram_tensor("clsh", [85, 1], F32, kind="ExternalInput")
    d_a1f = nc.dram_tensor("a1f", [425, 425], F32R, kind="ExternalInput")
    d_a1h = nc.dram_tensor("a1h", [85, 272], F32R, kind="ExternalInput")
    d_a1v = nc.dram_tensor("a1v", [85, 272], F32R, kind="ExternalInput")
    d_w2f = nc.dram_tensor("w2f", [425, 320], F32R, kind="ExternalInput")
    d_w2h = nc.dram_tensor("w2h", [272, 64], F32R, kind="ExternalInput")
    d_w2v = nc.dram_tensor("w2v", [272, 64], F32R, kind="ExternalInput")
    d_w3 = nc.dram_tensor("w3", [961, OF], BF16, kind="ExternalInput")
    d_b1f = nc.dram_tensor("b1f", [128, 4], F32, kind="ExternalInput")
    d_b1h = nc.dram_tensor("b1h", [128, 3], F32, kind="ExternalInput")
    d_b1v = nc.dram_tensor("b1v", [128, 3], F32, kind="ExternalInput")
    d_b2f = nc.dram_tensor("b2f", [128, 3], F32, kind="ExternalInput")
    d_b2h = nc.dram_tensor("b2h", [64, 1], F32, kind="ExternalInput")
    d_b2v = nc.dram_tensor("b2v", [64, 1], F32, kind="ExternalInput")
    d_y = nc.dram_tensor("y", [BC, OF], F32, kind="ExternalOutput")

    with tile.TileContext(nc) as tc:
        with (
            tc.tile_pool(name="const", bufs=1) as cp,
            tc.tile_pool(name="work", bufs=2) as wp,
            tc.tile_pool(name="oh", bufs=3) as ohp,
            tc.tile_pool(name="outp", bufs=3) as op_,
            tc.tile_pool(name="ps_s", bufs=4, space="PSUM") as pp,
            tc.tile_pool(name="ps_l3", bufs=1, space="PSUM") as pp3,
        ):
            # ---- load constants/weights into SBUF ----
            xt = cp.tile([NC_, BC], BF16, tag="xt")
            nc.sync.dma_start(xt[:], d_xt[:])
            sf = cp.tile([NC_, 425], BF16, tag="sf")
            nc.sync.dma_start(sf[:], d_sf[:])
            sh = cp.tile([NC_, 425], BF16, tag="sh")
            nc.sync.dma_start(sh[:], d_sh[:])
            sv = cp.tile([NC_, 425], BF16, tag="sv")
            nc.sync.dma_start(sv[:], d_sv[:])
            clsf = cp.tile([128, 4], F32, tag="clsf")
            nc.sync.dma_start(clsf[:], d_clsf[:])
            clsh = cp.tile([85, 1], F32, tag="clsh")
            nc.sync.dma_start(clsh[:], d_clsh[:])

            a1f = []
            for kc, (k0, kp) in enumerate(FULL_CH):
                t = cp.tile([kp, 425], F32R, tag=f"a1f_{kc}")
                nc.sync.dma_start(t[:], d_a1f[k0:k0 + kp, :])
                a1f.append(t)
            a1h = cp.tile([85, 272], F32R, tag="a1h")
            nc.sync.dma_start(a1h[:], d_a1h[:])
            a1v = cp.tile([85, 272], F32R, tag="a1v")
            nc.sync.dma_start(a1v[:], d_a1v[:])

            w2f = []
            for kc, (k0, kp) in enumerate(FULL_CH):
                t = cp.tile([kp, 320], F32R, tag=f"w2f_{kc}")
                nc.sync.dma_start(t[:], d_w2f[k0:k0 + kp, :])
                w2f.append(t)
            w2h = []
            w2v = []
            for kc, (k0, kp) in enumerate(H_CH):
                t = cp.tile([kp, 64], F32R, tag=f"w2h_{kc}")
                nc.sync.dma_start(t[:], d_w2h[k0:k0 + kp, :])
                w2h.append(t)
                t = cp.tile([kp, 64], F32R, tag=f"w2v_{kc}")
                nc.sync.dma_start(t[:], d_w2v[k0:k0 + kp, :])
                w2v.append(t)
            w3 = []
            r0 = 0
            for i, sz in enumerate(A2_SIZES):
                t = cp.tile([sz, OF], BF16, tag=f"w3_{i}")
                nc.sync.dma_start(t[:], d_w3[r0:r0 + sz, :])
                w3.append(t)
                r0 += sz

            b1f = cp.tile([128, 4], F32, tag="b1f")
            nc.sync.dma_start(b1f[:], d_b1f[:])
            b1h = cp.tile([128, 3], F32, tag="b1h")
            nc.sync.dma_start(b1h[:], d_b1h[:])
            b1v = cp.tile([128, 3], F32, tag="b1v")
            nc.sync.dma_start(b1v[:], d_b1v[:])
            b2f = cp.tile([128, 3], F32, tag="b2f")
            nc.sync.dma_start(b2f[:], d_b2f[:])
            b2h = cp.tile([64, 1], F32, tag="b2h")
            nc.sync.dma_start(b2h[:], d_b2h[:])
            b2v = cp.tile([64, 1], F32, tag="b2v")
            nc.sync.dma_start(b2v[:], d_b2v[:])

            # ---- batch-tile pipeline ----
            for t_i in range(NTILES):
                n0 = t_i * NT
                xs = xt[:, n0:n0 + NT]

                A2 = [wp.tile([A2_SIZES[i], NT], BF16, tag=f"a2_{i}",
                              name=f"a2_{i}_{t_i}")
                      for i in range(8)]

                # ===== full branch =====
                ohf = []
                for kc, (k0, kp) in enumerate(FULL_CH):
                    ps = pp.tile([kp, NT], F32, tag="ps_s")
                    nc.tensor.matmul(ps[:], sf[:, k0:k0 + kp], xs,
                                     start=True, stop=True)
                    oht = ohp.tile([kp, NT], F32R, tag=f"ohf{kc}")
                    nc.vector.tensor_scalar(oht[:], ps[:],
                                            clsf[0:kp, kc:kc + 1], None,
                                            op0=EQ)
                    ohf.append(oht)

                act1f = []
                for mc, (m0, mp) in enumerate(FULL_CH):
                    ps = pp.tile([mp, NT], F32, tag="ps_s")
                    kcs = FULL_BLOCKS[mc]
                    for i, kc in enumerate(kcs):
                        nc.tensor.matmul(ps[:], a1f[kc][:, m0:m0 + mp],
                                         ohf[kc][:],
                                         start=(i == 0),
                                         stop=(i == len(kcs) - 1))
                    a = wp.tile([mp, NT], F32R, tag=f"act1f{mc}")
                    nc.scalar.activation(a[:], ps[:], LRELU,
                                         bias=b1f[0:mp, mc:mc + 1],
                                         alpha=SLOPE)
                    act1f.append(a)

                for mc2, (m0, mp) in enumerate(F2_CH):
                    ps = pp.tile([mp, NT], F32, tag="ps_s")
                    for i in range(4):
                        nc.tensor.matmul(ps[:], w2f[i][:, m0:m0 + mp],
                                         act1f[i][:],
                                         start=(i == 0), stop=(i == 3))
                    if mc2 < 2:
                        dst = A2[mc2][0:128, :]
                    else:
                        dst = A2[2][0:64, :]
                    nc.scalar.activation(dst, ps[:], LRELU,
                                         bias=b2f[0:mp, mc2:mc2 + 1],
                                         alpha=SLOPE)

                # ===== hori / vert branches =====
                for branch, (s_mat, a1_mat, w2_t, b1_t, b2_t, dst_map) in (
                    ("h", (sh, a1h, w2h, b1h, b2h, H_DST)),
                    ("v", (sv, a1v, w2v, b1v, b2v, V_DST)),
                ):
                    for r in range(5):
                        ps = pp.tile([85, NT], F32, tag="ps_s")
                        nc.tensor.matmul(ps[:], s_mat[:, 85 * r:85 * r + 85],
                                         xs, start=True, stop=True)
                        ohr = ohp.tile([85, NT], F32R, tag=f"oh{branch}")
                        nc.vector.tensor_scalar(ohr[:], ps[:],
                                                clsh[:, 0:1], None, op0=EQ)

                        a1_t = []
                        for mc, (m0, mp) in enumerate(H_CH):
                            ps1 = pp.tile([mp, NT], F32, tag="ps_s")
                            nc.tensor.matmul(ps1[:], a1_mat[:, m0:m0 + mp],
                                             ohr[:], start=True, stop=True)
                            a = wp.tile([mp, NT], F32R,
                                        tag=f"act1{branch}{mc}")
                            nc.scalar.activation(a[:], ps1[:], LRELU,
                                                 bias=b1_t[0:mp, mc:mc + 1],
                                                 alpha=SLOPE)
                            a1_t.append(a)

                        ps2 = pp.tile([64, NT], F32, tag="ps_s")
                        for i, (m0, mp) in enumerate(H_CH):
                            nc.tensor.matmul(ps2[:], w2_t[i][:, 0:64],
                                             a1_t[i][:],
                                             start=(i == 0), stop=(i == 2))
                        ti, off = dst_map[r]
                        nc.scalar.activation(A2[ti][off:off + 64, :], ps2[:],
                                             LRELU, bias=b2_t[0:64, 0:1],
                                             alpha=SLOPE)

                # ones row for the bias of the output layer
                nc.vector.memset(A2[7][64:65, :], 1.0)

                # ===== output layer (batch on partitions) =====
                for q in range(4):
                    b0 = q * 128
                    psA = pp3.tile([128, 1024], F32, tag="ps_l3a")
                    psB = pp3.tile([128, 576], F32, tag="ps_l3b")
                    for i in range(8):
                        lh = A2[i][:, b0:b0 + 128]
                        st, sp_ = (i == 0), (i == 7)
                        nc.tensor.matmul(psA[:, 0:512], lh,
                                         w3[i][:, 0:512], start=st, stop=sp_)
                        nc.tensor.matmul(psA[:, 512:1024], lh,
                                         w3[i][:, 512:1024], start=st, stop=sp_)
                        nc.tensor.matmul(psB[:, 0:512], lh,
                                         w3[i][:, 1024:1536], start=st, stop=sp_)
                        nc.tensor.matmul(psB[:, 512:576], lh,
                                         w3[i][:, 1536:1600], start=st, stop=sp_)
                    o = op_.tile([128, OF], F32, tag="outt")
                    nc.scalar.activation(o[:, 0:1024], psA[:], LRELU,
                                         alpha=SLOPE)
                    nc.scalar.activation(o[:, 1024:1600], psB[:], LRELU,
                                         alpha=SLOPE)
                    nc.sync.dma_start(d_y[n0 + b0:n0 + b0 + 128, :], o[:])

    nc.compile()
    return nc


_NC_CACHE = None


def _get_nc():
    global _NC_CACHE
    if _NC_CACHE is None:
        _NC_CACHE = _build_nc()
    return _NC_CACHE


def _prep_weights(inputs):
    W_df = np.asarray(inputs["W_df"], dtype=np.float32)
    b_df = np.asarray(inputs["b_df"], dtype=np.float32)
    W_pf = np.asarray(inputs["W_pf"], dtype=np.float32)
    b_pf = np.asarray(inputs["b_pf"], dtype=np.float32)
    W_dh = np.asarray(inputs["W_dh"], dtype=np.float32)
    b_dh = np.asarray(inputs["b_dh"], dtype=np.float32)
    W_ph = np.asarray(inputs["W_ph"], dtype=np.float32)
    b_ph = np.asarray(inputs["b_ph"], dtype=np.float32)
    W_dv = np.asarray(inputs["W_dv"], dtype=np.float32)
    b_dv = np.asarray(inputs["b_dv"], dtype=np.float32)
    W_pv = np.asarray(inputs["W_pv"], dtype=np.float32)
    b_pv = np.asarray(inputs["b_pv"], dtype=np.float32)
    W_out = np.asarray(inputs["W_out"], dtype=np.float32)
    b_out = np.asarray(inputs["b_out"], dtype=np.float32)

    cc = np.arange(NCL)
    ll = np.arange(NC_)

    A_full = np.zeros((425, 425), np.float32)
    for c in range(NCL):
        # block [l, m] = W_df[c, m, l]
        A_full[25 * c:25 * c + 25, 25 * c:25 * c + 25] = W_df[c].T
    A_h = np.zeros((85, 272), np.float32)
    A_v = np.zeros((85, 272), np.float32)
    for c in range(NCL):
        A_h[5 * c:5 * c + 5, 16 * c:16 * c + 16] = W_dh[c].T  # [j, m]
        A_v[5 * c:5 * c + 5, 16 * c:16 * c + 16] = W_dv[c].T  # [r, m]

    # selection (broadcast) matrices, bf16-exact 0/1
    p = np.arange(425)
    sf = (ll[:, None] == (p % 25)[None, :]).astype(BF16NP)
    # sh: col 85*r + 5*c + j -> row l = 5*r + j
    sh = np.zeros((NC_, 425), BF16NP)
    # sv: col 85*j + 5*c + r -> row l = 5*r + j
    sv = np.zeros((NC_, 425), BF16NP)
    for c in range(NCL):
        for r in range(5):
            for j in range(5):
                sh[5 * r + j, 85 * r + 5 * c + j] = 1
                sv[5 * r + j, 85 * j + 5 * c + r] = 1

    clsf = np.zeros((128, 4), np.float32)
    for kc, (k0, kp) in enumerate(FULL_CH):
        clsf[0:kp, kc] = ((k0 + np.arange(kp)) // 25).astype(np.float32)
    clsh = (np.arange(85) // 5).astype(np.float32)[:, None]

    # output-layer weights, rows reordered to the act2 chunk layout
    W3re = np.zeros((961, OF), np.float32)
    W3re[0:320] = W_out[:, :, 0:5].transpose(1, 2, 0).reshape(320, OF)
    W3re[320:640] = W_out[:, :, 5:10].transpose(2, 1, 0).reshape(320, OF)
    W3re[640:960] = W_out[:, :, 10:15].transpose(2, 1, 0).reshape(320, OF)
    W3re[960] = b_out

    b1f = np.zeros((128, 4), np.float32)
    for mc, (m0, mp) in enumerate(FULL_CH):
        b1f[0:mp, mc] = b_df[m0:m0 + mp]
    b1h = np.zeros((128, 3), np.float32)
    b1v = np.zeros((128, 3), np.float32)
    for mc, (m0, mp) in enumerate(H_CH):
        b1h[0:mp, mc] = b_dh[m0:m0 + mp]
        b1v[0:mp, mc] = b_dv[m0:m0 + mp]
    b2f = np.zeros((128, 3), np.float32)
    for mc, (m0, mp) in enumerate(F2_CH):
        b2f[0:mp, mc] = b_pf[m0:m0 + mp]

    return {
        "sf": sf, "sh": sh, "sv": sv,
        "clsf": clsf, "clsh": clsh,
        "a1f": A_full, "a1h": A_h, "a1v": A_v,
        "w2f": np.ascontiguousarray(W_pf.T),
        "w2h": np.ascontiguousarray(W_ph.T),
        "w2v": np.ascontiguousarray(W_pv.T),
        "w3": W3re.astype(BF16NP),
        "b1f": b1f, "b1h": b1h, "b1v": b1v,
        "b2f": b2f,
        "b2h": b_ph[:, None].copy(),
        "b2v": b_pv[:, None].copy(),
    }


def kernel(**inputs) -> np.ndarray:
    x = np.asarray(inputs["x"]).astype(np.int32)
    assert x.shape == (B_FULL, NC_), x.shape

    shared = _prep_weights(inputs)
    nc = _get_nc()

    in_maps = []
    for core in range(NCORES):
        xs = x[core * BC:(core + 1) * BC]          # [BC, 25]
        xtc = np.ascontiguousarray(xs.T).astype(BF16NP)  # [25, BC]
        m = dict(shared)
        m["xt"] = xtc
        in_maps.append(m)

    res = run_bass_kernel_spmd(nc, in_maps, core_ids=list(range(NCORES)))
    global LAST_RESULTS
    LAST_RESULTS = res
    out = np.concatenate([res.results[i]["y"] for i in range(NCORES)], axis=0)
    return out


LAST_RESULTS = None


# revision 6
# speedup vs baseline: 1.2411x; 1.2411x over previous
"""Trainium2 Bass kernel for nn_NewCNNEncoder (dense CNN encoder over one-hot boards).

Strategy (pure data parallel over 8 NeuronCores, 8192 samples each):
  - The input x [B, 25] (values 0..16) is one-hot encoded ON CHIP via
    broadcast-matmul + is_equal compare, in three layouts matched to the
    three depthwise-conv branches (full / horizontal / vertical).
  - All convolutions are expressed as dense matmuls with activations kept
    in [features-on-partitions, batch-free] layout; the final conv_out
    layer flips to [batch-on-partitions, features-free] so the output DMA
    is contiguous.
  - Matmuls run in float32r (full-rate fp32 mode) except the first-layer
    broadcast and the last layer, which run in bf16.
  - leaky_relu(+bias) epilogues are single ScalarE activation ops reading
    PSUM directly.
"""

import sys

sys.path.insert(0, "/opt/trn_rl_repo")

import numpy as np
import ml_dtypes

import concourse.mybir as mybir
import concourse.tile as tile
from concourse import bacc
from concourse.bass_utils import run_bass_kernel_spmd

NCORES = 8
B_FULL = 65536
BC = B_FULL // NCORES  # 8192 per core
NT = 512               # batch tile (samples per pipeline tile)
NTILES = BC // NT      # 16

NC_ = 25   # cells (5x5 board)
NCL = 17   # classes
MULT = 16
OC = 64
OF = 1600
SLOPE = 0.01

F32 = mybir.dt.float32
F32R = mybir.dt.float32r
BF16 = mybir.dt.bfloat16
BF16NP = ml_dtypes.bfloat16
LRELU = mybir.ActivationFunctionType.Lrelu
EQ = mybir.AluOpType.is_equal

# one-hot row chunking, 425 rows (p = 25c + l) zero-padded to 512
FULL_CH = [(0, 128), (128, 128), (256, 128), (384, 128)]
# per-r (and per-j) L1 output col chunks, 272 (16c + m) zero-padded to 384
H_CH = [(0, 128), (128, 128), (256, 128)]
# L2-full output chunks over 320
F2_CH = [(0, 128), (128, 128), (256, 64)]
# act2 (cat) K-chunk sizes: 7x128 + 65 (last = vert j4 (64) + ones row)
A2_SIZES = [128] * 7 + [65]
# where each L2 output block lands in the A2 tiles: branch -> (tile, part_off)
H_DST = {0: (2, 64), 1: (3, 0), 2: (3, 64), 3: (4, 0), 4: (4, 64)}
V_DST = {0: (5, 0), 1: (5, 64), 2: (6, 0), 3: (6, 64), 4: (7, 0)}


def _full_blocks():
    """For each full-branch M-chunk, the list of one-hot K-chunks with a
    nonzero block of the block-diagonal depthwise matrix."""
    out = []
    for m0, mp in FULL_CH:
        hi = min(m0 + mp, 425)  # cols beyond 425 are zero padding
        c_lo, c_hi = m0 // 25, (hi - 1) // 25
        r_lo, r_hi = 25 * c_lo, 25 * c_hi + 25
        kcs = [kc for kc, (k0, kp) in enumerate(FULL_CH)
               if k0 < r_hi and k0 + kp > r_lo]
        out.append(kcs)
    return out


FULL_BLOCKS = _full_blocks()


def _build_nc():
    nc = bacc.Bacc("TRN2", target_bir_lowering=False, debug=False)

    # ---- DRAM I/O ----
    d_xt = nc.dram_tensor("xt", [128, BC], BF16, kind="ExternalInput")
    d_sf = nc.dram_tensor("sf", [128, 512], BF16, kind="ExternalInput")
    d_sh = nc.dram_tensor("sh", [128, 425], BF16, kind="ExternalInput")
    d_sv = nc.dram_tensor("sv", [128, 425], BF16, kind="ExternalInput")
    d_clsf = nc.dram_tensor("clsf", [128, 4], F32, kind="ExternalInput")
    d_clsh = nc.dram_tensor("clsh", [85, 1], F32, kind="ExternalInput")
    d_a1f = nc.dram_tensor("a1f", [512, 512], F32R, kind="ExternalInput")
    d_a1h = nc.dram_tensor("a1h", [85, 384], F32R, kind="ExternalInput")
    d_a1v = nc.dram_tensor("a1v", [85, 384], F32R, kind="ExternalInput")
    d_w2f = nc.dram_tensor("w2f", [512, 320], F32R, kind="ExternalInput")
    d_w2h = nc.dram_tensor("w2h", [384, 64], F32R, kind="ExternalInput")
    d_w2v = nc.dram_tensor("w2v", [384, 64], F32R, kind="ExternalInput")
    d_w3 = nc.dram_tensor("w3", [961, OF], BF16, kind="ExternalInput")
    d_b1f = nc.dram_tensor("b1f", [128, 4], F32, kind="ExternalInput")
    d_b1h = nc.dram_tensor("b1h", [128, 3], F32, kind="ExternalInput")
    d_b1v = nc.dram_tensor("b1v", [128, 3], F32, kind="ExternalInput")
    d_b2f = nc.dram_tensor("b2f", [128, 3], F32, kind="ExternalInput")
    d_b2h = nc.dram_tensor("b2h", [64, 1], F32, kind="ExternalInput")
    d_b2v = nc.dram_tensor("b2v", [64, 1], F32, kind="ExternalInput")
    d_y = nc.dram_tensor("y", [BC, OF], F32, kind="ExternalOutput")

    with tile.TileContext(nc) as tc:
        with (
            tc.tile_pool(name="const", bufs=1) as cp,
            tc.tile_pool(name="work", bufs=2) as wp,
            tc.tile_pool(name="oh", bufs=3) as ohp,
            tc.tile_pool(name="outp", bufs=3) as op_,
            tc.tile_pool(name="ps_s", bufs=4, space="PSUM") as pp,
            tc.tile_pool(name="ps_l3", bufs=1, space="PSUM") as pp3,
        ):
            # ---- load constants/weights into SBUF ----
            xt = cp.tile([128, BC], BF16, tag="xt")
            nc.sync.dma_start(xt[:], d_xt[:])
            sf = cp.tile([128, 512], BF16, tag="sf")
            nc.sync.dma_start(sf[:], d_sf[:])
            sh = cp.tile([128, 425], BF16, tag="sh")
            nc.sync.dma_start(sh[:], d_sh[:])
            sv = cp.tile([128, 425], BF16, tag="sv")
            nc.sync.dma_start(sv[:], d_sv[:])
            clsf = cp.tile([128, 4], F32, tag="clsf")
            nc.sync.dma_start(clsf[:], d_clsf[:])
            clsh = cp.tile([85, 1], F32, tag="clsh")
            nc.sync.dma_start(clsh[:], d_clsh[:])

            a1f = []
            for kc, (k0, kp) in enumerate(FULL_CH):
                t = cp.tile([kp, 512], F32R, tag=f"a1f_{kc}")
                nc.sync.dma_start(t[:], d_a1f[k0:k0 + kp, :])
                a1f.append(t)
            a1h = cp.tile([85, 384], F32R, tag="a1h")
            nc.sync.dma_start(a1h[:], d_a1h[:])
            a1v = cp.tile([85, 384], F32R, tag="a1v")
            nc.sync.dma_start(a1v[:], d_a1v[:])

            w2f = []
            for kc, (k0, kp) in enumerate(FULL_CH):
                t = cp.tile([kp, 320], F32R, tag=f"w2f_{kc}")
                nc.sync.dma_start(t[:], d_w2f[k0:k0 + kp, :])
                w2f.append(t)
            w2h = []
            w2v = []
            for kc, (k0, kp) in enumerate(H_CH):
                t = cp.tile([kp, 64], F32R, tag=f"w2h_{kc}")
                nc.sync.dma_start(t[:], d_w2h[k0:k0 + kp, :])
                w2h.append(t)
                t = cp.tile([kp, 64], F32R, tag=f"w2v_{kc}")
                nc.sync.dma_start(t[:], d_w2v[k0:k0 + kp, :])
                w2v.append(t)
            w3 = []
            r0 = 0
            for i, sz in enumerate(A2_SIZES):
                t = cp.tile([sz, OF], BF16, tag=f"w3_{i}")
                nc.sync.dma_start(t[:], d_w3[r0:r0 + sz, :])
                w3.append(t)
                r0 += sz

            b1f = cp.tile([128, 4], F32, tag="b1f")
            nc.sync.dma_start(b1f[:], d_b1f[:])
            b1h = cp.tile([128, 3], F32, tag="b1h")
            nc.sync.dma_start(b1h[:], d_b1h[:])
            b1v = cp.tile([128, 3], F32, tag="b1v")
            nc.sync.dma_start(b1v[:], d_b1v[:])
            b2f = cp.tile([128, 3], F32, tag="b2f")
            nc.sync.dma_start(b2f[:], d_b2f[:])
            b2h = cp.tile([64, 1], F32, tag="b2h")
            nc.sync.dma_start(b2h[:], d_b2h[:])
            b2v = cp.tile([64, 1], F32, tag="b2v")
            nc.sync.dma_start(b2v[:], d_b2v[:])

            # ---- batch-tile pipeline ----
            for t_i in range(NTILES):
                n0 = t_i * NT
                xs = xt[:, n0:n0 + NT]

                A2 = [wp.tile([A2_SIZES[i], NT], BF16, tag=f"a2_{i}",
                              name=f"a2_{i}_{t_i}")
                      for i in range(8)]

                # ===== full branch =====
                ohf = []
                for kc, (k0, kp) in enumerate(FULL_CH):
                    ps = pp.tile([kp, NT], F32, tag="ps_s")
                    nc.tensor.matmul(ps[:], sf[:, k0:k0 + kp], xs,
                                     start=True, stop=True)
                    oht = ohp.tile([kp, NT], F32R, tag=f"ohf{kc}")
                    nc.vector.tensor_scalar(oht[:], ps[:],
                                            clsf[0:kp, kc:kc + 1], None,
                                            op0=EQ)
                    ohf.append(oht)

                act1f = []
                for mc, (m0, mp) in enumerate(FULL_CH):
                    ps = pp.tile([mp, NT], F32, tag="ps_s")
                    kcs = FULL_BLOCKS[mc]
                    for i, kc in enumerate(kcs):
                        nc.tensor.matmul(ps[:], a1f[kc][:, m0:m0 + mp],
                                         ohf[kc][:],
                                         start=(i == 0),
                                         stop=(i == len(kcs) - 1))
                    a = wp.tile([mp, NT], F32R, tag=f"act1f{mc}")
                    nc.scalar.activation(a[:], ps[:], LRELU,
                                         bias=b1f[0:mp, mc:mc + 1],
                                         alpha=SLOPE)
                    act1f.append(a)

                for mc2, (m0, mp) in enumerate(F2_CH):
                    ps = pp.tile([mp, NT], F32, tag="ps_s")
                    for i in range(4):
                        nc.tensor.matmul(ps[:], w2f[i][:, m0:m0 + mp],
                                         act1f[i][:],
                                         start=(i == 0), stop=(i == 3))
                    if mc2 < 2:
                        dst = A2[mc2][0:128, :]
                    else:
                        dst = A2[2][0:64, :]
                    nc.scalar.activation(dst, ps[:], LRELU,
                                         bias=b2f[0:mp, mc2:mc2 + 1],
                                         alpha=SLOPE)

                # ===== hori / vert branches =====
                for branch, (s_mat, a1_mat, w2_t, b1_t, b2_t, dst_map) in (
                    ("h", (sh, a1h, w2h, b1h, b2h, H_DST)),
                    ("v", (sv, a1v, w2v, b1v, b2v, V_DST)),
                ):
                    for r in range(5):
                        ps = pp.tile([85, NT], F32, tag="ps_s")
                        nc.tensor.matmul(ps[:], s_mat[:, 85 * r:85 * r + 85],
                                         xs, start=True, stop=True)
                        ohr = ohp.tile([85, NT], F32R, tag=f"oh{branch}")
                        nc.vector.tensor_scalar(ohr[:], ps[:],
                                                clsh[:, 0:1], None, op0=EQ)

                        a1_t = []
                        for mc, (m0, mp) in enumerate(H_CH):
                            ps1 = pp.tile([mp, NT], F32, tag="ps_s")
                            nc.tensor.matmul(ps1[:], a1_mat[:, m0:m0 + mp],
                                             ohr[:], start=True, stop=True)
                            a = wp.tile([mp, NT], F32R,
                                        tag=f"act1{branch}{mc}")
                            nc.scalar.activation(a[:], ps1[:], LRELU,
                                                 bias=b1_t[0:mp, mc:mc + 1],
                                                 alpha=SLOPE)
                            a1_t.append(a)

                        ps2 = pp.tile([64, NT], F32, tag="ps_s")
                        for i, (m0, mp) in enumerate(H_CH):
                            nc.tensor.matmul(ps2[:], w2_t[i][:, 0:64],
                                             a1_t[i][:],
                                             start=(i == 0), stop=(i == 2))
                        ti, off = dst_map[r]
                        nc.scalar.activation(A2[ti][off:off + 64, :], ps2[:],
                                             LRELU, bias=b2_t[0:64, 0:1],
                                             alpha=SLOPE)

                # ones row for the bias of the output layer
                nc.vector.memset(A2[7][64:65, :], 1.0)

                # ===== output layer (batch on partitions) =====
                for q in range(4):
                    b0 = q * 128
                    psA = pp3.tile([128, 1024], F32, tag="ps_l3a")
                    psB = pp3.tile([128, 576], F32, tag="ps_l3b")
                    for i in range(8):
                        lh = A2[i][:, b0:b0 + 128]
                        st, sp_ = (i == 0), (i == 7)
                        nc.tensor.matmul(psA[:, 0:512], lh,
                                         w3[i][:, 0:512], start=st, stop=sp_)
                        nc.tensor.matmul(psA[:, 512:1024], lh,
                                         w3[i][:, 512:1024], start=st, stop=sp_)
                        nc.tensor.matmul(psB[:, 0:512], lh,
                                         w3[i][:, 1024:1536], start=st, stop=sp_)
                        nc.tensor.matmul(psB[:, 512:576], lh,
                                         w3[i][:, 1536:1600], start=st, stop=sp_)
                    o = op_.tile([128, OF], F32, tag="outt")
                    nc.scalar.activation(o[:, 0:1024], psA[:], LRELU,
                                         alpha=SLOPE)
                    nc.scalar.activation(o[:, 1024:1600], psB[:], LRELU,
                                         alpha=SLOPE)
                    nc.sync.dma_start(d_y[n0 + b0:n0 + b0 + 128, :], o[:])

    nc.compile()
    return nc


_NC_CACHE = None


def _get_nc():
    global _NC_CACHE
    if _NC_CACHE is None:
        _NC_CACHE = _build_nc()
    return _NC_CACHE


def _prep_weights(inputs):
    W_df = np.asarray(inputs["W_df"], dtype=np.float32)
    b_df = np.asarray(inputs["b_df"], dtype=np.float32)
    W_pf = np.asarray(inputs["W_pf"], dtype=np.float32)
    b_pf = np.asarray(inputs["b_pf"], dtype=np.float32)
    W_dh = np.asarray(inputs["W_dh"], dtype=np.float32)
    b_dh = np.asarray(inputs["b_dh"], dtype=np.float32)
    W_ph = np.asarray(inputs["W_ph"], dtype=np.float32)
    b_ph = np.asarray(inputs["b_ph"], dtype=np.float32)
    W_dv = np.asarray(inputs["W_dv"], dtype=np.float32)
    b_dv = np.asarray(inputs["b_dv"], dtype=np.float32)
    W_pv = np.asarray(inputs["W_pv"], dtype=np.float32)
    b_pv = np.asarray(inputs["b_pv"], dtype=np.float32)
    W_out = np.asarray(inputs["W_out"], dtype=np.float32)
    b_out = np.asarray(inputs["b_out"], dtype=np.float32)

    cc = np.arange(NCL)
    ll = np.arange(NC_)

    A_full = np.zeros((512, 512), np.float32)
    for c in range(NCL):
        # block [l, m] = W_df[c, m, l]
        A_full[25 * c:25 * c + 25, 25 * c:25 * c + 25] = W_df[c].T
    A_h = np.zeros((85, 384), np.float32)
    A_v = np.zeros((85, 384), np.float32)
    for c in range(NCL):
        A_h[5 * c:5 * c + 5, 16 * c:16 * c + 16] = W_dh[c].T  # [j, m]
        A_v[5 * c:5 * c + 5, 16 * c:16 * c + 16] = W_dv[c].T  # [r, m]

    # selection (broadcast) matrices, bf16-exact 0/1 (K padded 25 -> 128)
    p = np.arange(425)
    sf = np.zeros((128, 512), BF16NP)
    sf[:NC_, :425] = (ll[:, None] == (p % 25)[None, :]).astype(BF16NP)
    # sh: col 85*r + 5*c + j -> row l = 5*r + j
    sh = np.zeros((128, 425), BF16NP)
    # sv: col 85*j + 5*c + r -> row l = 5*r + j
    sv = np.zeros((128, 425), BF16NP)
    for c in range(NCL):
        for r in range(5):
            for j in range(5):
                sh[5 * r + j, 85 * r + 5 * c + j] = 1
                sv[5 * r + j, 85 * j + 5 * c + r] = 1

    # class constant per one-hot row; -1 on padding rows (matches nothing)
    clsf = np.full((128, 4), -1.0, np.float32)
    for kc, (k0, kp) in enumerate(FULL_CH):
        valid = max(0, min(kp, 425 - k0))
        clsf[0:valid, kc] = ((k0 + np.arange(valid)) // 25).astype(np.float32)
    clsh = (np.arange(85) // 5).astype(np.float32)[:, None]

    # output-layer weights, rows reordered to the act2 chunk layout
    W3re = np.zeros((961, OF), np.float32)
    W3re[0:320] = W_out[:, :, 0:5].transpose(1, 2, 0).reshape(320, OF)
    W3re[320:640] = W_out[:, :, 5:10].transpose(2, 1, 0).reshape(320, OF)
    W3re[640:960] = W_out[:, :, 10:15].transpose(2, 1, 0).reshape(320, OF)
    W3re[960] = b_out

    b1f = np.zeros((128, 4), np.float32)
    for mc, (m0, mp) in enumerate(FULL_CH):
        valid = max(0, min(mp, 425 - m0))
        b1f[0:valid, mc] = b_df[m0:m0 + valid]
    b1h = np.zeros((128, 3), np.float32)
    b1v = np.zeros((128, 3), np.float32)
    for mc, (m0, mp) in enumerate(H_CH):
        valid = max(0, min(mp, 272 - m0))
        b1h[0:valid, mc] = b_dh[m0:m0 + valid]
        b1v[0:valid, mc] = b_dv[m0:m0 + valid]
    b2f = np.zeros((128, 3), np.float32)
    for mc, (m0, mp) in enumerate(F2_CH):
        b2f[0:mp, mc] = b_pf[m0:m0 + mp]

    w2f_p = np.zeros((512, 320), np.float32)
    w2f_p[0:425] = W_pf.T
    w2h_p = np.zeros((384, 64), np.float32)
    w2h_p[0:272] = W_ph.T
    w2v_p = np.zeros((384, 64), np.float32)
    w2v_p[0:272] = W_pv.T

    return {
        "sf": sf, "sh": sh, "sv": sv,
        "clsf": clsf, "clsh": clsh,
        "a1f": A_full, "a1h": A_h, "a1v": A_v,
        "w2f": w2f_p, "w2h": w2h_p, "w2v": w2v_p,
        "w3": W3re.astype(BF16NP),
        "b1f": b1f, "b1h": b1h, "b1v": b1v,
        "b2f": b2f,
        "b2h": b_ph[:, None].copy(),
        "b2v": b_pv[:, None].copy(),
    }


def kernel(**inputs) -> np.ndarray:
    x = np.asarray(inputs["x"]).astype(np.int32)
    assert x.shape == (B_FULL, NC_), x.shape

    shared = _prep_weights(inputs)
    nc = _get_nc()

    in_maps = []
    for core in range(NCORES):
        xs = x[core * BC:(core + 1) * BC]          # [BC, 25]
        xtc = np.zeros((128, BC), BF16NP)
        xtc[:NC_] = xs.T.astype(BF16NP)
        m = dict(shared)
        m["xt"] = xtc
        in_maps.append(m)

    res = run_bass_kernel_spmd(nc, in_maps, core_ids=list(range(NCORES)))
    global LAST_RESULTS
    LAST_RESULTS = res
    out = np.concatenate([res.results[i]["y"] for i in range(NCORES)], axis=0)
    return out


LAST_RESULTS = None


# revision 8
# speedup vs baseline: 1.2539x; 1.0103x over previous
"""Trainium2 Bass kernel for nn_NewCNNEncoder (dense CNN encoder over one-hot boards).

Strategy (pure data parallel over 8 NeuronCores, 8192 samples each):
  - The input x [B, 25] (values 0..16) is one-hot encoded ON CHIP via
    broadcast-matmul + is_equal compare, in three layouts matched to the
    three depthwise-conv branches (full / horizontal / vertical).
  - All convolutions are expressed as dense matmuls with activations kept
    in [features-on-partitions, batch-free] layout; the final conv_out
    layer flips to [batch-on-partitions, features-free] so the output DMA
    is contiguous.
  - Matmuls run in float32r (full-rate fp32 mode) except the first-layer
    broadcast and the last layer, which run in bf16.
  - leaky_relu(+bias) epilogues are single ScalarE activation ops reading
    PSUM directly.
"""

import sys

sys.path.insert(0, "/opt/trn_rl_repo")

import numpy as np
import ml_dtypes

import concourse.mybir as mybir
import concourse.tile as tile
from concourse import bacc
from concourse.bass_utils import run_bass_kernel_spmd

NCORES = 8
B_FULL = 65536
BC = B_FULL // NCORES  # 8192 per core
NT = 512               # batch tile (samples per pipeline tile)
NTILES = BC // NT      # 16

NC_ = 25   # cells (5x5 board)
NCL = 17   # classes
MULT = 16
OC = 64
OF = 1600
SLOPE = 0.01

F32 = mybir.dt.float32
F32R = mybir.dt.float32r
BF16 = mybir.dt.bfloat16
BF16NP = ml_dtypes.bfloat16
LRELU = mybir.ActivationFunctionType.Lrelu
EQ = mybir.AluOpType.is_equal

# one-hot row chunking, 425 rows (p = 25c + l) zero-padded to 512
FULL_CH = [(0, 128), (128, 128), (256, 128), (384, 128)]
# per-r (and per-j) L1 output col chunks, 272 (16c + m) zero-padded to 384
H_CH = [(0, 128), (128, 128), (256, 128)]
# L2-full output chunks over 320
F2_CH = [(0, 128), (128, 128), (256, 64)]
# act2 (cat) K-chunk sizes: 7x128 + 65 (last = vert j4 (64) + ones row)
A2_SIZES = [128] * 7 + [65]
# where each L2 output block lands in the A2 tiles: branch -> (tile, part_off)
H_DST = {0: (2, 64), 1: (3, 0), 2: (3, 64), 3: (4, 0), 4: (4, 64)}
V_DST = {0: (5, 0), 1: (5, 64), 2: (6, 0), 3: (6, 64), 4: (7, 0)}


def _fr(c, l):
    """class-aligned padded row/feature index for (class, cell)"""
    return 128 * (c // 5) + 25 * (c % 5) + l


def _build_nc():
    nc = bacc.Bacc("TRN2", target_bir_lowering=False, debug=False)

    # ---- DRAM I/O ----
    d_xt = nc.dram_tensor("xt", [128, BC], BF16, kind="ExternalInput")
    d_sf = nc.dram_tensor("sf", [128, 512], BF16, kind="ExternalInput")
    d_sh = nc.dram_tensor("sh", [128, 425], BF16, kind="ExternalInput")
    d_sv = nc.dram_tensor("sv", [128, 425], BF16, kind="ExternalInput")
    d_clsf = nc.dram_tensor("clsf", [128, 4], F32, kind="ExternalInput")
    d_clsh = nc.dram_tensor("clsh", [85, 1], F32, kind="ExternalInput")
    d_a1f = nc.dram_tensor("a1f", [512, 512], F32R, kind="ExternalInput")
    d_a1h = nc.dram_tensor("a1h", [85, 384], F32R, kind="ExternalInput")
    d_a1v = nc.dram_tensor("a1v", [85, 384], F32R, kind="ExternalInput")
    d_w2f = nc.dram_tensor("w2f", [512, 320], F32R, kind="ExternalInput")
    d_w2h = nc.dram_tensor("w2h", [384, 64], F32R, kind="ExternalInput")
    d_w2v = nc.dram_tensor("w2v", [384, 64], F32R, kind="ExternalInput")
    d_w3 = nc.dram_tensor("w3", [961, OF], BF16, kind="ExternalInput")
    d_b1f = nc.dram_tensor("b1f", [128, 4], F32, kind="ExternalInput")
    d_b1h = nc.dram_tensor("b1h", [128, 3], F32, kind="ExternalInput")
    d_b1v = nc.dram_tensor("b1v", [128, 3], F32, kind="ExternalInput")
    d_b2f = nc.dram_tensor("b2f", [128, 3], F32, kind="ExternalInput")
    d_b2h = nc.dram_tensor("b2h", [64, 1], F32, kind="ExternalInput")
    d_b2v = nc.dram_tensor("b2v", [64, 1], F32, kind="ExternalInput")
    d_y = nc.dram_tensor("y", [BC, OF], F32, kind="ExternalOutput")

    with tile.TileContext(nc) as tc:
        with (
            tc.tile_pool(name="const", bufs=1) as cp,
            tc.tile_pool(name="work", bufs=2) as wp,
            tc.tile_pool(name="oh", bufs=3) as ohp,
            tc.tile_pool(name="outp", bufs=3) as op_,
            tc.tile_pool(name="ps_s", bufs=4, space="PSUM") as pp,
            tc.tile_pool(name="ps_l3", bufs=1, space="PSUM") as pp3,
        ):
            # ---- load constants/weights into SBUF ----
            xt = cp.tile([128, BC], BF16, tag="xt")
            for t_i in range(NTILES):
                nc.sync.dma_start(xt[:, t_i * NT:(t_i + 1) * NT],
                                  d_xt[:, t_i * NT:(t_i + 1) * NT])
            sf = cp.tile([128, 512], BF16, tag="sf")
            nc.sync.dma_start(sf[:], d_sf[:])
            sh = cp.tile([128, 425], BF16, tag="sh")
            nc.sync.dma_start(sh[:], d_sh[:])
            sv = cp.tile([128, 425], BF16, tag="sv")
            nc.sync.dma_start(sv[:], d_sv[:])
            clsf = cp.tile([128, 4], F32, tag="clsf")
            nc.sync.dma_start(clsf[:], d_clsf[:])
            clsh = cp.tile([85, 1], F32, tag="clsh")
            nc.sync.dma_start(clsh[:], d_clsh[:])

            a1f = []
            for kc, (k0, kp) in enumerate(FULL_CH):
                t = cp.tile([kp, 512], F32R, tag=f"a1f_{kc}")
                nc.sync.dma_start(t[:], d_a1f[k0:k0 + kp, :])
                a1f.append(t)
            a1h = cp.tile([85, 384], F32R, tag="a1h")
            nc.sync.dma_start(a1h[:], d_a1h[:])
            a1v = cp.tile([85, 384], F32R, tag="a1v")
            nc.sync.dma_start(a1v[:], d_a1v[:])

            w2f = []
            for kc, (k0, kp) in enumerate(FULL_CH):
                t = cp.tile([kp, 320], F32R, tag=f"w2f_{kc}")
                nc.sync.dma_start(t[:], d_w2f[k0:k0 + kp, :])
                w2f.append(t)
            w2h = []
            w2v = []
            for kc, (k0, kp) in enumerate(H_CH):
                t = cp.tile([kp, 64], F32R, tag=f"w2h_{kc}")
                nc.sync.dma_start(t[:], d_w2h[k0:k0 + kp, :])
                w2h.append(t)
                t = cp.tile([kp, 64], F32R, tag=f"w2v_{kc}")
                nc.sync.dma_start(t[:], d_w2v[k0:k0 + kp, :])
                w2v.append(t)
            w3 = []
            r0 = 0
            for i, sz in enumerate(A2_SIZES):
                t = cp.tile([sz, OF], BF16, tag=f"w3_{i}")
                nc.sync.dma_start(t[:], d_w3[r0:r0 + sz, :])
                w3.append(t)
                r0 += sz

            b1f = cp.tile([128, 4], F32, tag="b1f")
            nc.sync.dma_start(b1f[:], d_b1f[:])
            b1h = cp.tile([128, 3], F32, tag="b1h")
            nc.sync.dma_start(b1h[:], d_b1h[:])
            b1v = cp.tile([128, 3], F32, tag="b1v")
            nc.sync.dma_start(b1v[:], d_b1v[:])
            b2f = cp.tile([128, 3], F32, tag="b2f")
            nc.sync.dma_start(b2f[:], d_b2f[:])
            b2h = cp.tile([64, 1], F32, tag="b2h")
            nc.sync.dma_start(b2h[:], d_b2h[:])
            b2v = cp.tile([64, 1], F32, tag="b2v")
            nc.sync.dma_start(b2v[:], d_b2v[:])

            # ---- batch-tile pipeline ----
            for t_i in range(NTILES):
                n0 = t_i * NT
                xs = xt[:, n0:n0 + NT]

                A2 = [wp.tile([A2_SIZES[i], NT], BF16, tag=f"a2_{i}",
                              name=f"a2_{i}_{t_i}")
                      for i in range(8)]

                # ===== full branch =====
                ohf = []
                for kc, (k0, kp) in enumerate(FULL_CH):
                    ps = pp.tile([kp, NT], F32, tag="ps_s")
                    nc.tensor.matmul(ps[:], sf[:, k0:k0 + kp], xs,
                                     start=True, stop=True)
                    oht = ohp.tile([kp, NT], F32R, tag=f"ohf{kc}")
                    nc.vector.tensor_scalar(oht[:], ps[:],
                                            clsf[0:kp, kc:kc + 1], None,
                                            op0=EQ)
                    ohf.append(oht)

                act1f = []
                for mc, (m0, mp) in enumerate(FULL_CH):
                    ps = pp.tile([mp, NT], F32, tag="ps_s")
                    nc.tensor.matmul(ps[:], a1f[mc][:, m0:m0 + mp],
                                     ohf[mc][:], start=True, stop=True)
                    a = wp.tile([mp, NT], F32R, tag=f"act1f{mc}")
                    nc.scalar.activation(a[:], ps[:], LRELU,
                                         bias=b1f[0:mp, mc:mc + 1],
                                         alpha=SLOPE)
                    act1f.append(a)

                for mc2, (m0, mp) in enumerate(F2_CH):
                    ps = pp.tile([mp, NT], F32, tag="ps_s")
                    for i in range(4):
                        nc.tensor.matmul(ps[:], w2f[i][:, m0:m0 + mp],
                                         act1f[i][:],
                                         start=(i == 0), stop=(i == 3))
                    if mc2 < 2:
                        dst = A2[mc2][0:128, :]
                    else:
                        dst = A2[2][0:64, :]
                    nc.scalar.activation(dst, ps[:], LRELU,
                                         bias=b2f[0:mp, mc2:mc2 + 1],
                                         alpha=SLOPE)

                # ===== hori / vert branches =====
                for branch, (s_mat, a1_mat, w2_t, b1_t, b2_t, dst_map) in (
                    ("h", (sh, a1h, w2h, b1h, b2h, H_DST)),
                    ("v", (sv, a1v, w2v, b1v, b2v, V_DST)),
                ):
                    for r in range(5):
                        ps = pp.tile([85, NT], F32, tag="ps_s")
                        nc.tensor.matmul(ps[:], s_mat[:, 85 * r:85 * r + 85],
                                         xs, start=True, stop=True)
                        ohr = ohp.tile([85, NT], F32R, tag=f"oh{branch}")
                        nc.vector.tensor_scalar(ohr[:], ps[:],
                                                clsh[:, 0:1], None, op0=EQ)

                        a1_t = []
                        for mc, (m0, mp) in enumerate(H_CH):
                            ps1 = pp.tile([mp, NT], F32, tag="ps_s")
                            nc.tensor.matmul(ps1[:], a1_mat[:, m0:m0 + mp],
                                             ohr[:], start=True, stop=True)
                            a = wp.tile([mp, NT], F32R,
                                        tag=f"act1{branch}{mc}")
                            nc.scalar.activation(a[:], ps1[:], LRELU,
                                                 bias=b1_t[0:mp, mc:mc + 1],
                                                 alpha=SLOPE)
                            a1_t.append(a)

                        ps2 = pp.tile([64, NT], F32, tag="ps_s")
                        for i, (m0, mp) in enumerate(H_CH):
                            nc.tensor.matmul(ps2[:], w2_t[i][:, 0:64],
                                             a1_t[i][:],
                                             start=(i == 0), stop=(i == 2))
                        ti, off = dst_map[r]
                        nc.scalar.activation(A2[ti][off:off + 64, :], ps2[:],
                                             LRELU, bias=b2_t[0:64, 0:1],
                                             alpha=SLOPE)

                # ones row for the bias of the output layer
                nc.vector.memset(A2[7][64:65, :], 1.0)

                # ===== output layer (batch on partitions) =====
                for q in range(4):
                    b0 = q * 128
                    psA = pp3.tile([128, 1024], F32, tag="ps_l3a")
                    psB = pp3.tile([128, 576], F32, tag="ps_l3b")
                    for i in range(8):
                        lh = A2[i][:, b0:b0 + 128]
                        st, sp_ = (i == 0), (i == 7)
                        nc.tensor.matmul(psA[:, 0:512], lh,
                                         w3[i][:, 0:512], start=st, stop=sp_)
                        nc.tensor.matmul(psA[:, 512:1024], lh,
                                         w3[i][:, 512:1024], start=st, stop=sp_)
                        nc.tensor.matmul(psB[:, 0:512], lh,
                                         w3[i][:, 1024:1536], start=st, stop=sp_)
                        nc.tensor.matmul(psB[:, 512:576], lh,
                                         w3[i][:, 1536:1600], start=st, stop=sp_)
                    o = op_.tile([128, OF], F32, tag="outt")
                    nc.scalar.activation(o[:, 0:1024], psA[:], LRELU,
                                         alpha=SLOPE)
                    nc.scalar.activation(o[:, 1024:1600], psB[:], LRELU,
                                         alpha=SLOPE)
                    nc.sync.dma_start(d_y[n0 + b0:n0 + b0 + 128, :], o[:])

    nc.compile()
    return nc


_NC_CACHE = None


def _get_nc():
    global _NC_CACHE
    if _NC_CACHE is None:
        _NC_CACHE = _build_nc()
    return _NC_CACHE


def _prep_weights(inputs):
    W_df = np.asarray(inputs["W_df"], dtype=np.float32)
    b_df = np.asarray(inputs["b_df"], dtype=np.float32)
    W_pf = np.asarray(inputs["W_pf"], dtype=np.float32)
    b_pf = np.asarray(inputs["b_pf"], dtype=np.float32)
    W_dh = np.asarray(inputs["W_dh"], dtype=np.float32)
    b_dh = np.asarray(inputs["b_dh"], dtype=np.float32)
    W_ph = np.asarray(inputs["W_ph"], dtype=np.float32)
    b_ph = np.asarray(inputs["b_ph"], dtype=np.float32)
    W_dv = np.asarray(inputs["W_dv"], dtype=np.float32)
    b_dv = np.asarray(inputs["b_dv"], dtype=np.float32)
    W_pv = np.asarray(inputs["W_pv"], dtype=np.float32)
    b_pv = np.asarray(inputs["b_pv"], dtype=np.float32)
    W_out = np.asarray(inputs["W_out"], dtype=np.float32)
    b_out = np.asarray(inputs["b_out"], dtype=np.float32)

    cc = np.arange(NCL)
    ll = np.arange(NC_)

    A_full = np.zeros((512, 512), np.float32)
    for c in range(NCL):
        r0, c0 = _fr(c, 0), _fr(c, 0)
        # block [l, m] = W_df[c, m, l]
        A_full[r0:r0 + 25, c0:c0 + 25] = W_df[c].T
    A_h = np.zeros((85, 384), np.float32)
    A_v = np.zeros((85, 384), np.float32)
    for c in range(NCL):
        A_h[5 * c:5 * c + 5, 16 * c:16 * c + 16] = W_dh[c].T  # [j, m]
        A_v[5 * c:5 * c + 5, 16 * c:16 * c + 16] = W_dv[c].T  # [r, m]

    # selection (broadcast) matrices, bf16-exact 0/1 (K padded 25 -> 128)
    sf = np.zeros((128, 512), BF16NP)
    for c in range(NCL):
        for l in range(NC_):
            sf[l, _fr(c, l)] = 1
    # sh: col 85*r + 5*c + j -> row l = 5*r + j
    sh = np.zeros((128, 425), BF16NP)
    # sv: col 85*j + 5*c + r -> row l = 5*r + j
    sv = np.zeros((128, 425), BF16NP)
    for c in range(NCL):
        for r in range(5):
            for j in range(5):
                sh[5 * r + j, 85 * r + 5 * c + j] = 1
                sv[5 * r + j, 85 * j + 5 * c + r] = 1

    # class constant per one-hot row; -1 on padding rows (matches nothing)
    clsf = np.full((128, 4), -1.0, np.float32)
    for kc in range(4):
        for c in range(5 * kc, min(5 * kc + 5, NCL)):
            p0 = 25 * (c % 5)
            clsf[p0:p0 + 25, kc] = float(c)
    clsh = (np.arange(85) // 5).astype(np.float32)[:, None]

    # output-layer weights, rows reordered to the act2 chunk layout
    W3re = np.zeros((961, OF), np.float32)
    W3re[0:320] = W_out[:, :, 0:5].transpose(1, 2, 0).reshape(320, OF)
    W3re[320:640] = W_out[:, :, 5:10].transpose(2, 1, 0).reshape(320, OF)
    W3re[640:960] = W_out[:, :, 10:15].transpose(2, 1, 0).reshape(320, OF)
    W3re[960] = b_out

    b1f = np.zeros((128, 4), np.float32)
    for mc in range(4):
        for c in range(5 * mc, min(5 * mc + 5, NCL)):
            p0 = 25 * (c % 5)
            b1f[p0:p0 + 25, mc] = b_df[25 * c:25 * c + 25]
    b1h = np.zeros((128, 3), np.float32)
    b1v = np.zeros((128, 3), np.float32)
    for mc, (m0, mp) in enumerate(H_CH):
        valid = max(0, min(mp, 272 - m0))
        b1h[0:valid, mc] = b_dh[m0:m0 + valid]
        b1v[0:valid, mc] = b_dv[m0:m0 + valid]
    b2f = np.zeros((128, 3), np.float32)
    for mc, (m0, mp) in enumerate(F2_CH):
        b2f[0:mp, mc] = b_pf[m0:m0 + mp]

    w2f_p = np.zeros((512, 320), np.float32)
    for c in range(NCL):
        r0 = _fr(c, 0)
        w2f_p[r0:r0 + 25] = W_pf.T[25 * c:25 * c + 25]
    w2h_p = np.zeros((384, 64), np.float32)
    w2h_p[0:272] = W_ph.T
    w2v_p = np.zeros((384, 64), np.float32)
    w2v_p[0:272] = W_pv.T

    return {
        "sf": sf, "sh": sh, "sv": sv,
        "clsf": clsf, "clsh": clsh,
        "a1f": A_full, "a1h": A_h, "a1v": A_v,
        "w2f": w2f_p, "w2h": w2h_p, "w2v": w2v_p,
        "w3": W3re.astype(BF16NP),
        "b1f": b1f, "b1h": b1h, "b1v": b1v,
        "b2f": b2f,
        "b2h": b_ph[:, None].copy(),
        "b2v": b_pv[:, None].copy(),
    }


def kernel(**inputs) -> np.ndarray:
    x = np.asarray(inputs["x"]).astype(np.int32)
    assert x.shape == (B_FULL, NC_), x.shape

    shared = _prep_weights(inputs)
    nc = _get_nc()

    in_maps = []
    for core in range(NCORES):
        xs = x[core * BC:(core + 1) * BC]          # [BC, 25]
        xtc = np.zeros((128, BC), BF16NP)
        xtc[:NC_] = xs.T.astype(BF16NP)
        m = dict(shared)
        m["xt"] = xtc
        in_maps.append(m)

    res = run_bass_kernel_spmd(nc, in_maps, core_ids=list(range(NCORES)))
    global LAST_RESULTS
    LAST_RESULTS = res
    out = np.concatenate([res.results[i]["y"] for i in range(NCORES)], axis=0)
    return out


LAST_RESULTS = None


# revision 9
# speedup vs baseline: 1.2796x; 1.0205x over previous
"""Trainium2 Bass kernel for nn_NewCNNEncoder (dense CNN encoder over one-hot boards).

Strategy (pure data parallel over 8 NeuronCores, 8192 samples each):
  - The input x [B, 25] (values 0..16) is one-hot encoded ON CHIP via
    broadcast-matmul + is_equal compare, in three layouts matched to the
    three depthwise-conv branches (full / horizontal / vertical).
  - All convolutions are expressed as dense matmuls with activations kept
    in [features-on-partitions, batch-free] layout; the final conv_out
    layer flips to [batch-on-partitions, features-free] so the output DMA
    is contiguous.
  - Matmuls run in float32r (full-rate fp32 mode) except the first-layer
    broadcast and the last layer, which run in bf16.
  - leaky_relu(+bias) epilogues are single ScalarE activation ops reading
    PSUM directly.
"""

import sys

sys.path.insert(0, "/opt/trn_rl_repo")

import numpy as np
import ml_dtypes

import concourse.mybir as mybir
import concourse.tile as tile
from concourse import bacc
from concourse.bass_utils import run_bass_kernel_spmd

NCORES = 8
B_FULL = 65536
BC = B_FULL // NCORES  # 8192 per core
NT = 512               # batch tile (samples per pipeline tile)
NTILES = BC // NT      # 16

NC_ = 25   # cells (5x5 board)
NCL = 17   # classes
MULT = 16
OC = 64
OF = 1600
SLOPE = 0.01

F32 = mybir.dt.float32
F32R = mybir.dt.float32r
BF16 = mybir.dt.bfloat16
BF16NP = ml_dtypes.bfloat16
LRELU = mybir.ActivationFunctionType.Lrelu
EQ = mybir.AluOpType.is_equal

# one-hot row chunking, 425 rows (p = 25c + l) zero-padded to 512
FULL_CH = [(0, 128), (128, 128), (256, 128), (384, 128)]
# per-r (and per-j) L1 output col chunks, 272 (16c + m) zero-padded to 384
H_CH = [(0, 128), (128, 128), (256, 128)]
# L2-full output chunks over 320
F2_CH = [(0, 128), (128, 128), (256, 64)]
# act2 (cat) K-chunk sizes: 7x128 + 65 (last = vert j4 (64) + ones row)
A2_SIZES = [128] * 7 + [65]
# where each L2 output block lands in the A2 tiles: branch -> (tile, part_off)
H_DST = {0: (2, 64), 1: (3, 0), 2: (3, 64), 3: (4, 0), 4: (4, 64)}
V_DST = {0: (5, 0), 1: (5, 64), 2: (6, 0), 3: (6, 64), 4: (7, 0)}


def _fr(c, l):
    """class-aligned padded row/feature index for (class, cell)"""
    return 128 * (c // 5) + 25 * (c % 5) + l


def _build_nc():
    nc = bacc.Bacc("TRN2", target_bir_lowering=False, debug=False)

    # ---- DRAM I/O ----
    d_xt = nc.dram_tensor("xt", [128, BC], BF16, kind="ExternalInput")
    d_sf = nc.dram_tensor("sf", [128, 512], BF16, kind="ExternalInput")
    d_sh = nc.dram_tensor("sh", [128, 425], BF16, kind="ExternalInput")
    d_sv = nc.dram_tensor("sv", [128, 425], BF16, kind="ExternalInput")
    d_clsf = nc.dram_tensor("clsf", [128, 4], F32, kind="ExternalInput")
    d_clsh = nc.dram_tensor("clsh", [85, 1], F32, kind="ExternalInput")
    d_a1f = nc.dram_tensor("a1f", [512, 512], F32R, kind="ExternalInput")
    d_a1h = nc.dram_tensor("a1h", [85, 384], F32R, kind="ExternalInput")
    d_a1v = nc.dram_tensor("a1v", [85, 384], F32R, kind="ExternalInput")
    d_w2f = nc.dram_tensor("w2f", [512, 320], F32R, kind="ExternalInput")
    d_w2h = nc.dram_tensor("w2h", [384, 64], F32R, kind="ExternalInput")
    d_w2v = nc.dram_tensor("w2v", [384, 64], F32R, kind="ExternalInput")
    d_w3 = nc.dram_tensor("w3", [961, OF], BF16, kind="ExternalInput")
    d_b1f = nc.dram_tensor("b1f", [128, 4], F32, kind="ExternalInput")
    d_b1h = nc.dram_tensor("b1h", [128, 3], F32, kind="ExternalInput")
    d_b1v = nc.dram_tensor("b1v", [128, 3], F32, kind="ExternalInput")
    d_b2f = nc.dram_tensor("b2f", [128, 3], F32, kind="ExternalInput")
    d_b2h = nc.dram_tensor("b2h", [64, 1], F32, kind="ExternalInput")
    d_b2v = nc.dram_tensor("b2v", [64, 1], F32, kind="ExternalInput")
    d_y = nc.dram_tensor("y", [BC, OF], F32, kind="ExternalOutput")

    with tile.TileContext(nc) as tc:
        with (
            tc.tile_pool(name="const", bufs=1) as cp,
            tc.tile_pool(name="work", bufs=2) as wp,
            tc.tile_pool(name="oh", bufs=3) as ohp,
            tc.tile_pool(name="outp", bufs=3) as op_,
            tc.tile_pool(name="ps_s", bufs=4, space="PSUM") as pp,
            tc.tile_pool(name="ps_l3", bufs=1, space="PSUM") as pp3,
        ):
            # ---- load constants/weights into SBUF ----
            xt = cp.tile([128, BC], BF16, tag="xt")
            for t_i in range(NTILES):
                nc.sync.dma_start(xt[:, t_i * NT:(t_i + 1) * NT],
                                  d_xt[:, t_i * NT:(t_i + 1) * NT])
            sf = cp.tile([128, 512], BF16, tag="sf")
            nc.sync.dma_start(sf[:], d_sf[:])
            sh = cp.tile([128, 425], BF16, tag="sh")
            nc.sync.dma_start(sh[:], d_sh[:])
            sv = cp.tile([128, 425], BF16, tag="sv")
            nc.sync.dma_start(sv[:], d_sv[:])
            clsf = cp.tile([128, 4], F32, tag="clsf")
            nc.sync.dma_start(clsf[:], d_clsf[:])
            clsh = cp.tile([85, 1], F32, tag="clsh")
            nc.sync.dma_start(clsh[:], d_clsh[:])

            a1f = []
            for kc, (k0, kp) in enumerate(FULL_CH):
                t = cp.tile([kp, 512], F32R, tag=f"a1f_{kc}")
                nc.sync.dma_start(t[:], d_a1f[k0:k0 + kp, :])
                a1f.append(t)
            a1h = cp.tile([85, 384], F32R, tag="a1h")
            nc.sync.dma_start(a1h[:], d_a1h[:])
            a1v = cp.tile([85, 384], F32R, tag="a1v")
            nc.sync.dma_start(a1v[:], d_a1v[:])

            w2f = []
            for kc, (k0, kp) in enumerate(FULL_CH):
                t = cp.tile([kp, 320], F32R, tag=f"w2f_{kc}")
                nc.sync.dma_start(t[:], d_w2f[k0:k0 + kp, :])
                w2f.append(t)
            w2h = []
            w2v = []
            for kc, (k0, kp) in enumerate(H_CH):
                t = cp.tile([kp, 64], F32R, tag=f"w2h_{kc}")
                nc.sync.dma_start(t[:], d_w2h[k0:k0 + kp, :])
                w2h.append(t)
                t = cp.tile([kp, 64], F32R, tag=f"w2v_{kc}")
                nc.sync.dma_start(t[:], d_w2v[k0:k0 + kp, :])
                w2v.append(t)
            w3 = []
            r0 = 0
            for i, sz in enumerate(A2_SIZES):
                t = cp.tile([sz, OF], BF16, tag=f"w3_{i}")
                nc.sync.dma_start(t[:], d_w3[r0:r0 + sz, :])
                w3.append(t)
                r0 += sz

            b1f = cp.tile([128, 4], F32, tag="b1f")
            nc.sync.dma_start(b1f[:], d_b1f[:])
            b1h = cp.tile([128, 3], F32, tag="b1h")
            nc.sync.dma_start(b1h[:], d_b1h[:])
            b1v = cp.tile([128, 3], F32, tag="b1v")
            nc.sync.dma_start(b1v[:], d_b1v[:])
            b2f = cp.tile([128, 3], F32, tag="b2f")
            nc.sync.dma_start(b2f[:], d_b2f[:])
            b2h = cp.tile([64, 1], F32, tag="b2h")
            nc.sync.dma_start(b2h[:], d_b2h[:])
            b2v = cp.tile([64, 1], F32, tag="b2v")
            nc.sync.dma_start(b2v[:], d_b2v[:])

            # ---- batch-tile pipeline ----
            for t_i in range(NTILES):
                n0 = t_i * NT
                xs = xt[:, n0:n0 + NT]

                A2 = [wp.tile([A2_SIZES[i], NT], BF16, tag=f"a2_{i}",
                              name=f"a2_{i}_{t_i}")
                      for i in range(8)]

                # ===== full branch =====
                ohf = []
                for kc, (k0, kp) in enumerate(FULL_CH):
                    ps = pp.tile([kp, NT], F32, tag="ps_s")
                    nc.tensor.matmul(ps[:], sf[:, k0:k0 + kp], xs,
                                     start=True, stop=True)
                    oht = ohp.tile([kp, NT], F32R, tag=f"ohf{kc}")
                    nc.vector.tensor_scalar(oht[:], ps[:],
                                            clsf[0:kp, kc:kc + 1], None,
                                            op0=EQ)
                    ohf.append(oht)

                act1f = []
                for mc, (m0, mp) in enumerate(FULL_CH):
                    ps = pp.tile([mp, NT], F32, tag="ps_s")
                    nc.tensor.matmul(ps[:], a1f[mc][:, m0:m0 + mp],
                                     ohf[mc][:], start=True, stop=True)
                    a = wp.tile([mp, NT], F32R, tag=f"act1f{mc}")
                    nc.scalar.activation(a[:], ps[:], LRELU,
                                         bias=b1f[0:mp, mc:mc + 1],
                                         alpha=SLOPE)
                    act1f.append(a)

                for mc2, (m0, mp) in enumerate(F2_CH):
                    ps = pp.tile([mp, NT], F32, tag="ps_s")
                    for i in range(4):
                        nc.tensor.matmul(ps[:], w2f[i][:, m0:m0 + mp],
                                         act1f[i][:],
                                         start=(i == 0), stop=(i == 3))
                    if mc2 < 2:
                        dst = A2[mc2][0:128, :]
                    else:
                        dst = A2[2][0:64, :]
                    nc.scalar.activation(dst, ps[:], LRELU,
                                         bias=b2f[0:mp, mc2:mc2 + 1],
                                         alpha=SLOPE)

                # ===== hori / vert branches (interleaved per r) =====
                BR = (
                    ("h", sh, a1h, w2h, b1h, b2h, H_DST),
                    ("v", sv, a1v, w2v, b1v, b2v, V_DST),
                )
                for r in range(5):
                    ohr_b = {}
                    for (branch, s_mat, a1_mat, w2_t, b1_t, b2_t, dst_map) in BR:
                        ps = pp.tile([85, NT], F32, tag="ps_s",
                                     name=f"psb_{branch}{r}_{t_i}")
                        nc.tensor.matmul(ps[:], s_mat[:, 85 * r:85 * r + 85],
                                         xs, start=True, stop=True)
                        ohr = ohp.tile([85, NT], F32R, tag=f"oh{branch}",
                                       name=f"oh{branch}{r}_{t_i}")
                        nc.vector.tensor_scalar(ohr[:], ps[:],
                                                clsh[:, 0:1], None, op0=EQ)
                        ohr_b[branch] = ohr
                    a1_b = {"h": [], "v": []}
                    for mc, (m0, mp) in enumerate(H_CH):
                        for (branch, s_mat, a1_mat, w2_t, b1_t, b2_t,
                             dst_map) in BR:
                            ps1 = pp.tile([mp, NT], F32, tag="ps_s",
                                          name=f"ps1_{branch}{r}{mc}_{t_i}")
                            nc.tensor.matmul(ps1[:], a1_mat[:, m0:m0 + mp],
                                             ohr_b[branch][:],
                                             start=True, stop=True)
                            a = wp.tile([mp, NT], F32R,
                                        tag=f"act1{branch}{mc}",
                                        name=f"act1{branch}{r}{mc}_{t_i}")
                            nc.scalar.activation(a[:], ps1[:], LRELU,
                                                 bias=b1_t[0:mp, mc:mc + 1],
                                                 alpha=SLOPE)
                            a1_b[branch].append(a)
                    for (branch, s_mat, a1_mat, w2_t, b1_t, b2_t, dst_map) in BR:
                        ps2 = pp.tile([64, NT], F32, tag="ps_s",
                                      name=f"ps2_{branch}{r}_{t_i}")
                        for i, (m0, mp) in enumerate(H_CH):
                            nc.tensor.matmul(ps2[:], w2_t[i][:, 0:64],
                                             a1_b[branch][i][:],
                                             start=(i == 0), stop=(i == 2))
                        ti, off = dst_map[r]
                        nc.scalar.activation(A2[ti][off:off + 64, :], ps2[:],
                                             LRELU, bias=b2_t[0:64, 0:1],
                                             alpha=SLOPE)

                # ones row for the bias of the output layer
                nc.vector.memset(A2[7][64:65, :], 1.0)

                # ===== output layer (batch on partitions) =====
                for q in range(4):
                    b0 = q * 128
                    ps3 = pp3.tile([128, OF], F32, tag="ps_l3")
                    for i in range(8):
                        lh = A2[i][:, b0:b0 + 128]
                        st, sp_ = (i == 0), (i == 7)
                        nc.tensor.matmul(ps3[:, 0:512], lh,
                                         w3[i][:, 0:512], start=st, stop=sp_)
                        nc.tensor.matmul(ps3[:, 512:1024], lh,
                                         w3[i][:, 512:1024], start=st, stop=sp_)
                        nc.tensor.matmul(ps3[:, 1024:1536], lh,
                                         w3[i][:, 1024:1536], start=st, stop=sp_)
                        nc.tensor.matmul(ps3[:, 1536:1600], lh,
                                         w3[i][:, 1536:1600], start=st, stop=sp_)
                    o = op_.tile([128, OF], F32, tag="outt")
                    nc.scalar.activation(o[:], ps3[:], LRELU, alpha=SLOPE)
                    nc.sync.dma_start(d_y[n0 + b0:n0 + b0 + 128, :], o[:])

    nc.compile()
    return nc


_NC_CACHE = None


def _get_nc():
    global _NC_CACHE
    if _NC_CACHE is None:
        _NC_CACHE = _build_nc()
    return _NC_CACHE


def _prep_weights(inputs):
    W_df = np.asarray(inputs["W_df"], dtype=np.float32)
    b_df = np.asarray(inputs["b_df"], dtype=np.float32)
    W_pf = np.asarray(inputs["W_pf"], dtype=np.float32)
    b_pf = np.asarray(inputs["b_pf"], dtype=np.float32)
    W_dh = np.asarray(inputs["W_dh"], dtype=np.float32)
    b_dh = np.asarray(inputs["b_dh"], dtype=np.float32)
    W_ph = np.asarray(inputs["W_ph"], dtype=np.float32)
    b_ph = np.asarray(inputs["b_ph"], dtype=np.float32)
    W_dv = np.asarray(inputs["W_dv"], dtype=np.float32)
    b_dv = np.asarray(inputs["b_dv"], dtype=np.float32)
    W_pv = np.asarray(inputs["W_pv"], dtype=np.float32)
    b_pv = np.asarray(inputs["b_pv"], dtype=np.float32)
    W_out = np.asarray(inputs["W_out"], dtype=np.float32)
    b_out = np.asarray(inputs["b_out"], dtype=np.float32)

    cc = np.arange(NCL)
    ll = np.arange(NC_)

    A_full = np.zeros((512, 512), np.float32)
    for c in range(NCL):
        r0, c0 = _fr(c, 0), _fr(c, 0)
        # block [l, m] = W_df[c, m, l]
        A_full[r0:r0 + 25, c0:c0 + 25] = W_df[c].T
    A_h = np.zeros((85, 384), np.float32)
    A_v = np.zeros((85, 384), np.float32)
    for c in range(NCL):
        A_h[5 * c:5 * c + 5, 16 * c:16 * c + 16] = W_dh[c].T  # [j, m]
        A_v[5 * c:5 * c + 5, 16 * c:16 * c + 16] = W_dv[c].T  # [r, m]

    # selection (broadcast) matrices, bf16-exact 0/1 (K padded 25 -> 128)
    sf = np.zeros((128, 512), BF16NP)
    for c in range(NCL):
        for l in range(NC_):
            sf[l, _fr(c, l)] = 1
    # sh: col 85*r + 5*c + j -> row l = 5*r + j
    sh = np.zeros((128, 425), BF16NP)
    # sv: col 85*j + 5*c + r -> row l = 5*r + j
    sv = np.zeros((128, 425), BF16NP)
    for c in range(NCL):
        for r in range(5):
            for j in range(5):
                sh[5 * r + j, 85 * r + 5 * c + j] = 1
                sv[5 * r + j, 85 * j + 5 * c + r] = 1

    # class constant per one-hot row; -1 on padding rows (matches nothing)
    clsf = np.full((128, 4), -1.0, np.float32)
    for kc in range(4):
        for c in range(5 * kc, min(5 * kc + 5, NCL)):
            p0 = 25 * (c % 5)
            clsf[p0:p0 + 25, kc] = float(c)
    clsh = (np.arange(85) // 5).astype(np.float32)[:, None]

    # output-layer weights, rows reordered to the act2 chunk layout
    W3re = np.zeros((961, OF), np.float32)
    W3re[0:320] = W_out[:, :, 0:5].transpose(1, 2, 0).reshape(320, OF)
    W3re[320:640] = W_out[:, :, 5:10].transpose(2, 1, 0).reshape(320, OF)
    W3re[640:960] = W_out[:, :, 10:15].transpose(2, 1, 0).reshape(320, OF)
    W3re[960] = b_out

    b1f = np.zeros((128, 4), np.float32)
    for mc in range(4):
        for c in range(5 * mc, min(5 * mc + 5, NCL)):
            p0 = 25 * (c % 5)
            b1f[p0:p0 + 25, mc] = b_df[25 * c:25 * c + 25]
    b1h = np.zeros((128, 3), np.float32)
    b1v = np.zeros((128, 3), np.float32)
    for mc, (m0, mp) in enumerate(H_CH):
        valid = max(0, min(mp, 272 - m0))
        b1h[0:valid, mc] = b_dh[m0:m0 + valid]
        b1v[0:valid, mc] = b_dv[m0:m0 + valid]
    b2f = np.zeros((128, 3), np.float32)
    for mc, (m0, mp) in enumerate(F2_CH):
        b2f[0:mp, mc] = b_pf[m0:m0 + mp]

    w2f_p = np.zeros((512, 320), np.float32)
    for c in range(NCL):
        r0 = _fr(c, 0)
        w2f_p[r0:r0 + 25] = W_pf.T[25 * c:25 * c + 25]
    w2h_p = np.zeros((384, 64), np.float32)
    w2h_p[0:272] = W_ph.T
    w2v_p = np.zeros((384, 64), np.float32)
    w2v_p[0:272] = W_pv.T

    return {
        "sf": sf, "sh": sh, "sv": sv,
        "clsf": clsf, "clsh": clsh,
        "a1f": A_full, "a1h": A_h, "a1v": A_v,
        "w2f": w2f_p, "w2h": w2h_p, "w2v": w2v_p,
        "w3": W3re.astype(BF16NP),
        "b1f": b1f, "b1h": b1h, "b1v": b1v,
        "b2f": b2f,
        "b2h": b_ph[:, None].copy(),
        "b2v": b_pv[:, None].copy(),
    }


def kernel(**inputs) -> np.ndarray:
    x = np.asarray(inputs["x"]).astype(np.int32)
    assert x.shape == (B_FULL, NC_), x.shape

    shared = _prep_weights(inputs)
    nc = _get_nc()

    in_maps = []
    for core in range(NCORES):
        xs = x[core * BC:(core + 1) * BC]          # [BC, 25]
        xtc = np.zeros((128, BC), BF16NP)
        xtc[:NC_] = xs.T.astype(BF16NP)
        m = dict(shared)
        m["xt"] = xtc
        in_maps.append(m)

    res = run_bass_kernel_spmd(nc, in_maps, core_ids=list(range(NCORES)))
    global LAST_RESULTS
    LAST_RESULTS = res
    out = np.concatenate([res.results[i]["y"] for i in range(NCORES)], axis=0)
    return out


LAST_RESULTS = None


# revision 11
# speedup vs baseline: 1.3112x; 1.0246x over previous
"""Trainium2 Bass kernel for nn_NewCNNEncoder (dense CNN encoder over one-hot boards).

Strategy (pure data parallel over 8 NeuronCores, 8192 samples each):
  - The input x [B, 25] (values 0..16) is one-hot encoded ON CHIP via
    broadcast-matmul + is_equal compare, in three layouts matched to the
    three depthwise-conv branches (full / horizontal / vertical).
  - All convolutions are expressed as dense matmuls with activations kept
    in [features-on-partitions, batch-free] layout; the final conv_out
    layer flips to [batch-on-partitions, features-free] so the output DMA
    is contiguous.
  - Matmuls run in float32r (full-rate fp32 mode) except the first-layer
    broadcast and the last layer, which run in bf16.
  - leaky_relu(+bias) epilogues are single ScalarE activation ops reading
    PSUM directly.
"""

import sys

sys.path.insert(0, "/opt/trn_rl_repo")

import numpy as np
import ml_dtypes

import concourse.mybir as mybir
import concourse.tile as tile
from concourse import bacc
from concourse.bass_utils import run_bass_kernel_spmd

NCORES = 8
B_FULL = 65536
BC = B_FULL // NCORES  # 8192 per core
NT = 512               # batch tile (samples per pipeline tile)
NTILES = BC // NT      # 16

NC_ = 25   # cells (5x5 board)
NCL = 17   # classes
MULT = 16
OC = 64
OF = 1600
SLOPE = 0.01

F32 = mybir.dt.float32
F32R = mybir.dt.float32r
BF16 = mybir.dt.bfloat16
BF16NP = ml_dtypes.bfloat16
LRELU = mybir.ActivationFunctionType.Lrelu
EQ = mybir.AluOpType.is_equal

# one-hot row chunking, 425 rows (p = 25c + l) zero-padded to 512
FULL_CH = [(0, 128), (128, 128), (256, 128), (384, 128)]
# per-r (and per-j) L1 output col chunks, 272 (16c + m) zero-padded to 384
H_CH = [(0, 128), (128, 128), (256, 128)]
# L2-full output chunks over 320
F2_CH = [(0, 128), (128, 128), (256, 64)]
# act2 (cat) K-chunk sizes: 7x128 + 65 (last = vert j4 (64) + ones row)
A2_SIZES = [128] * 7 + [65]
# where each L2 output block lands in the A2 tiles: branch -> (tile, part_off)
H_DST = {0: (2, 64), 1: (3, 0), 2: (3, 64), 3: (4, 0), 4: (4, 64)}
V_DST = {0: (5, 0), 1: (5, 64), 2: (6, 0), 3: (6, 64), 4: (7, 0)}


def _fr(c, l):
    """class-aligned padded row/feature index for (class, cell)"""
    return 128 * (c // 5) + 25 * (c % 5) + l


def _build_nc():
    nc = bacc.Bacc("TRN2", target_bir_lowering=False, debug=False)

    # ---- DRAM I/O ----
    d_xt = nc.dram_tensor("xt", [128, BC], BF16, kind="ExternalInput")
    d_sf = nc.dram_tensor("sf", [128, 512], BF16, kind="ExternalInput")
    d_sh = nc.dram_tensor("sh", [128, 425], BF16, kind="ExternalInput")
    d_sv = nc.dram_tensor("sv", [128, 425], BF16, kind="ExternalInput")
    d_clsf = nc.dram_tensor("clsf", [128, 4], F32, kind="ExternalInput")
    d_clsh = nc.dram_tensor("clsh", [85, 1], F32, kind="ExternalInput")
    d_a1f = nc.dram_tensor("a1f", [512, 512], F32R, kind="ExternalInput")
    d_a1h = nc.dram_tensor("a1h", [85, 384], F32R, kind="ExternalInput")
    d_a1v = nc.dram_tensor("a1v", [85, 384], F32R, kind="ExternalInput")
    d_w2f = nc.dram_tensor("w2f", [512, 320], F32R, kind="ExternalInput")
    d_w2h = nc.dram_tensor("w2h", [384, 64], F32R, kind="ExternalInput")
    d_w2v = nc.dram_tensor("w2v", [384, 64], F32R, kind="ExternalInput")
    d_w3 = nc.dram_tensor("w3", [961, OF], BF16, kind="ExternalInput")
    d_b1f = nc.dram_tensor("b1f", [128, 4], F32, kind="ExternalInput")
    d_b1h = nc.dram_tensor("b1h", [128, 3], F32, kind="ExternalInput")
    d_b1v = nc.dram_tensor("b1v", [128, 3], F32, kind="ExternalInput")
    d_b2f = nc.dram_tensor("b2f", [128, 3], F32, kind="ExternalInput")
    d_b2h = nc.dram_tensor("b2h", [64, 1], F32, kind="ExternalInput")
    d_b2v = nc.dram_tensor("b2v", [64, 1], F32, kind="ExternalInput")
    d_y = nc.dram_tensor("y", [BC, OF], F32, kind="ExternalOutput")

    with tile.TileContext(nc) as tc:
        with (
            tc.tile_pool(name="const", bufs=1) as cp,
            tc.tile_pool(name="work", bufs=2) as wp,
            tc.tile_pool(name="oh", bufs=3) as ohp,
            tc.tile_pool(name="outp", bufs=3) as op_,
            tc.tile_pool(name="ps_s", bufs=4, space="PSUM") as pp,
            tc.tile_pool(name="ps_l3", bufs=2, space="PSUM") as pp3,
        ):
            # ---- load constants/weights into SBUF ----
            xt = cp.tile([128, BC], BF16, tag="xt")
            for t_i in range(NTILES):
                nc.sync.dma_start(xt[:, t_i * NT:(t_i + 1) * NT],
                                  d_xt[:, t_i * NT:(t_i + 1) * NT])
            sf = cp.tile([128, 512], BF16, tag="sf")
            nc.sync.dma_start(sf[:], d_sf[:])
            sh = cp.tile([128, 425], BF16, tag="sh")
            nc.sync.dma_start(sh[:], d_sh[:])
            sv = cp.tile([128, 425], BF16, tag="sv")
            nc.sync.dma_start(sv[:], d_sv[:])
            clsf = cp.tile([128, 4], F32, tag="clsf")
            nc.sync.dma_start(clsf[:], d_clsf[:])
            clsh = cp.tile([85, 1], F32, tag="clsh")
            nc.sync.dma_start(clsh[:], d_clsh[:])

            a1f = []
            for kc, (k0, kp) in enumerate(FULL_CH):
                t = cp.tile([kp, 512], F32R, tag=f"a1f_{kc}")
                nc.sync.dma_start(t[:], d_a1f[k0:k0 + kp, :])
                a1f.append(t)
            a1h = cp.tile([85, 384], F32R, tag="a1h")
            nc.sync.dma_start(a1h[:], d_a1h[:])
            a1v = cp.tile([85, 384], F32R, tag="a1v")
            nc.sync.dma_start(a1v[:], d_a1v[:])

            w2f = []
            for kc, (k0, kp) in enumerate(FULL_CH):
                t = cp.tile([kp, 320], F32R, tag=f"w2f_{kc}")
                nc.sync.dma_start(t[:], d_w2f[k0:k0 + kp, :])
                w2f.append(t)
            w2h = []
            w2v = []
            for kc, (k0, kp) in enumerate(H_CH):
                t = cp.tile([kp, 64], F32R, tag=f"w2h_{kc}")
                nc.sync.dma_start(t[:], d_w2h[k0:k0 + kp, :])
                w2h.append(t)
                t = cp.tile([kp, 64], F32R, tag=f"w2v_{kc}")
                nc.sync.dma_start(t[:], d_w2v[k0:k0 + kp, :])
                w2v.append(t)
            w3 = []
            r0 = 0
            for i, sz in enumerate(A2_SIZES):
                t = cp.tile([sz, OF], BF16, tag=f"w3_{i}")
                nc.sync.dma_start(t[:], d_w3[r0:r0 + sz, :])
                w3.append(t)
                r0 += sz

            b1f = cp.tile([128, 4], F32, tag="b1f")
            nc.sync.dma_start(b1f[:], d_b1f[:])
            b1h = cp.tile([128, 3], F32, tag="b1h")
            nc.sync.dma_start(b1h[:], d_b1h[:])
            b1v = cp.tile([128, 3], F32, tag="b1v")
            nc.sync.dma_start(b1v[:], d_b1v[:])
            b2f = cp.tile([128, 3], F32, tag="b2f")
            nc.sync.dma_start(b2f[:], d_b2f[:])
            b2h = cp.tile([64, 1], F32, tag="b2h")
            nc.sync.dma_start(b2h[:], d_b2h[:])
            b2v = cp.tile([64, 1], F32, tag="b2v")
            nc.sync.dma_start(b2v[:], d_b2v[:])

            # ---- batch-tile pipeline ----
            for t_i in range(NTILES):
                n0 = t_i * NT
                xs = xt[:, n0:n0 + NT]

                A2 = [wp.tile([A2_SIZES[i], NT], BF16, tag=f"a2_{i}",
                              name=f"a2_{i}_{t_i}")
                      for i in range(8)]

                # ===== full branch =====
                ohf = []
                for kc, (k0, kp) in enumerate(FULL_CH):
                    ps = pp.tile([kp, NT], F32, tag="ps_s")
                    nc.tensor.matmul(ps[:], sf[:, k0:k0 + kp], xs,
                                     start=True, stop=True)
                    oht = ohp.tile([kp, NT], F32R, tag=f"ohf{kc}")
                    nc.vector.tensor_scalar(oht[:], ps[:],
                                            clsf[0:kp, kc:kc + 1], None,
                                            op0=EQ)
                    ohf.append(oht)

                act1f = []
                for mc, (m0, mp) in enumerate(FULL_CH):
                    ps = pp.tile([mp, NT], F32, tag="ps_s")
                    nc.tensor.matmul(ps[:], a1f[mc][:, m0:m0 + mp],
                                     ohf[mc][:], start=True, stop=True)
                    a = wp.tile([mp, NT], F32R, tag=f"act1f{mc}")
                    nc.scalar.activation(a[:], ps[:], LRELU,
                                         bias=b1f[0:mp, mc:mc + 1],
                                         alpha=SLOPE)
                    act1f.append(a)

                for mc2, (m0, mp) in enumerate(F2_CH):
                    ps = pp.tile([mp, NT], F32, tag="ps_s")
                    for i in range(4):
                        nc.tensor.matmul(ps[:], w2f[i][:, m0:m0 + mp],
                                         act1f[i][:],
                                         start=(i == 0), stop=(i == 3))
                    if mc2 < 2:
                        dst = A2[mc2][0:128, :]
                    else:
                        dst = A2[2][0:64, :]
                    nc.scalar.activation(dst, ps[:], LRELU,
                                         bias=b2f[0:mp, mc2:mc2 + 1],
                                         alpha=SLOPE)

                # ===== hori / vert branches (interleaved per r) =====
                BR = (
                    ("h", sh, a1h, w2h, b1h, b2h, H_DST),
                    ("v", sv, a1v, w2v, b1v, b2v, V_DST),
                )
                for r in range(5):
                    ohr_b = {}
                    for (branch, s_mat, a1_mat, w2_t, b1_t, b2_t, dst_map) in BR:
                        ps = pp.tile([85, NT], F32, tag="ps_s",
                                     name=f"psb_{branch}{r}_{t_i}")
                        nc.tensor.matmul(ps[:], s_mat[:, 85 * r:85 * r + 85],
                                         xs, start=True, stop=True)
                        ohr = ohp.tile([85, NT], F32R, tag=f"oh{branch}",
                                       name=f"oh{branch}{r}_{t_i}")
                        nc.vector.tensor_scalar(ohr[:], ps[:],
                                                clsh[:, 0:1], None, op0=EQ)
                        ohr_b[branch] = ohr
                    a1_b = {"h": [], "v": []}
                    for mc, (m0, mp) in enumerate(H_CH):
                        for (branch, s_mat, a1_mat, w2_t, b1_t, b2_t,
                             dst_map) in BR:
                            ps1 = pp.tile([mp, NT], F32, tag="ps_s",
                                          name=f"ps1_{branch}{r}{mc}_{t_i}")
                            nc.tensor.matmul(ps1[:], a1_mat[:, m0:m0 + mp],
                                             ohr_b[branch][:],
                                             start=True, stop=True)
                            a = wp.tile([mp, NT], F32R,
                                        tag=f"act1{branch}{mc}",
                                        name=f"act1{branch}{r}{mc}_{t_i}")
                            nc.scalar.activation(a[:], ps1[:], LRELU,
                                                 bias=b1_t[0:mp, mc:mc + 1],
                                                 alpha=SLOPE)
                            a1_b[branch].append(a)
                    for (branch, s_mat, a1_mat, w2_t, b1_t, b2_t, dst_map) in BR:
                        ps2 = pp.tile([64, NT], F32, tag="ps_s",
                                      name=f"ps2_{branch}{r}_{t_i}")
                        for i, (m0, mp) in enumerate(H_CH):
                            nc.tensor.matmul(ps2[:], w2_t[i][:, 0:64],
                                             a1_b[branch][i][:],
                                             start=(i == 0), stop=(i == 2))
                        ti, off = dst_map[r]
                        nc.scalar.activation(A2[ti][off:off + 64, :], ps2[:],
                                             LRELU, bias=b2_t[0:64, 0:1],
                                             alpha=SLOPE)

                # ones row for the bias of the output layer
                nc.vector.memset(A2[7][64:65, :], 1.0)

                # ===== output layer (batch on partitions) =====
                for q in range(4):
                    b0 = q * 128
                    o = op_.tile([128, OF], F32, tag="outt")
                    for half in range(2):
                        c0 = half * 800
                        ps3 = pp3.tile([128, 800], F32, tag="ps_l3",
                                       name=f"ps3_{q}{half}_{t_i}")
                        for i in range(8):
                            lh = A2[i][:, b0:b0 + 128]
                            st, sp_ = (i == 0), (i == 7)
                            nc.tensor.matmul(ps3[:, 0:512], lh,
                                             w3[i][:, c0:c0 + 512],
                                             start=st, stop=sp_)
                            nc.tensor.matmul(ps3[:, 512:800], lh,
                                             w3[i][:, c0 + 512:c0 + 800],
                                             start=st, stop=sp_)
                        nc.scalar.activation(o[:, c0:c0 + 800], ps3[:],
                                             LRELU, alpha=SLOPE)
                    nc.sync.dma_start(d_y[n0 + b0:n0 + b0 + 128, :], o[:])

    nc.compile()
    return nc


_NC_CACHE = None


def _get_nc():
    global _NC_CACHE
    if _NC_CACHE is None:
        _NC_CACHE = _build_nc()
    return _NC_CACHE


def _prep_weights(inputs):
    W_df = np.asarray(inputs["W_df"], dtype=np.float32)
    b_df = np.asarray(inputs["b_df"], dtype=np.float32)
    W_pf = np.asarray(inputs["W_pf"], dtype=np.float32)
    b_pf = np.asarray(inputs["b_pf"], dtype=np.float32)
    W_dh = np.asarray(inputs["W_dh"], dtype=np.float32)
    b_dh = np.asarray(inputs["b_dh"], dtype=np.float32)
    W_ph = np.asarray(inputs["W_ph"], dtype=np.float32)
    b_ph = np.asarray(inputs["b_ph"], dtype=np.float32)
    W_dv = np.asarray(inputs["W_dv"], dtype=np.float32)
    b_dv = np.asarray(inputs["b_dv"], dtype=np.float32)
    W_pv = np.asarray(inputs["W_pv"], dtype=np.float32)
    b_pv = np.asarray(inputs["b_pv"], dtype=np.float32)
    W_out = np.asarray(inputs["W_out"], dtype=np.float32)
    b_out = np.asarray(inputs["b_out"], dtype=np.float32)

    cc = np.arange(NCL)
    ll = np.arange(NC_)

    A_full = np.zeros((512, 512), np.float32)
    for c in range(NCL):
        r0, c0 = _fr(c, 0), _fr(c, 0)
        # block [l, m] = W_df[c, m, l]
        A_full[r0:r0 + 25, c0:c0 + 25] = W_df[c].T
    A_h = np.zeros((85, 384), np.float32)
    A_v = np.zeros((85, 384), np.float32)
    for c in range(NCL):
        A_h[5 * c:5 * c + 5, 16 * c:16 * c + 16] = W_dh[c].T  # [j, m]
        A_v[5 * c:5 * c + 5, 16 * c:16 * c + 16] = W_dv[c].T  # [r, m]

    # selection (broadcast) matrices, bf16-exact 0/1 (K padded 25 -> 128)
    sf = np.zeros((128, 512), BF16NP)
    for c in range(NCL):
        for l in range(NC_):
            sf[l, _fr(c, l)] = 1
    # sh: col 85*r + 5*c + j -> row l = 5*r + j
    sh = np.zeros((128, 425), BF16NP)
    # sv: col 85*j + 5*c + r -> row l = 5*r + j
    sv = np.zeros((128, 425), BF16NP)
    for c in range(NCL):
        for r in range(5):
            for j in range(5):
                sh[5 * r + j, 85 * r + 5 * c + j] = 1
                sv[5 * r + j, 85 * j + 5 * c + r] = 1

    # class constant per one-hot row; -1 on padding rows (matches nothing)
    clsf = np.full((128, 4), -1.0, np.float32)
    for kc in range(4):
        for c in range(5 * kc, min(5 * kc + 5, NCL)):
            p0 = 25 * (c % 5)
            clsf[p0:p0 + 25, kc] = float(c)
    clsh = (np.arange(85) // 5).astype(np.float32)[:, None]

    # output-layer weights, rows reordered to the act2 chunk layout
    W3re = np.zeros((961, OF), np.float32)
    W3re[0:320] = W_out[:, :, 0:5].transpose(1, 2, 0).reshape(320, OF)
    W3re[320:640] = W_out[:, :, 5:10].transpose(2, 1, 0).reshape(320, OF)
    W3re[640:960] = W_out[:, :, 10:15].transpose(2, 1, 0).reshape(320, OF)
    W3re[960] = b_out

    b1f = np.zeros((128, 4), np.float32)
    for mc in range(4):
        for c in range(5 * mc, min(5 * mc + 5, NCL)):
            p0 = 25 * (c % 5)
            b1f[p0:p0 + 25, mc] = b_df[25 * c:25 * c + 25]
    b1h = np.zeros((128, 3), np.float32)
    b1v = np.zeros((128, 3), np.float32)
    for mc, (m0, mp) in enumerate(H_CH):
        valid = max(0, min(mp, 272 - m0))
        b1h[0:valid, mc] = b_dh[m0:m0 + valid]
        b1v[0:valid, mc] = b_dv[m0:m0 + valid]
    b2f = np.zeros((128, 3), np.float32)
    for mc, (m0, mp) in enumerate(F2_CH):
        b2f[0:mp, mc] = b_pf[m0:m0 + mp]

    w2f_p = np.zeros((512, 320), np.float32)
    for c in range(NCL):
        r0 = _fr(c, 0)
        w2f_p[r0:r0 + 25] = W_pf.T[25 * c:25 * c + 25]
    w2h_p = np.zeros((384, 64), np.float32)
    w2h_p[0:272] = W_ph.T
    w2v_p = np.zeros((384, 64), np.float32)
    w2v_p[0:272] = W_pv.T

    return {
        "sf": sf, "sh": sh, "sv": sv,
        "clsf": clsf, "clsh": clsh,
        "a1f": A_full, "a1h": A_h, "a1v": A_v,
        "w2f": w2f_p, "w2h": w2h_p, "w2v": w2v_p,
        "w3": W3re.astype(BF16NP),
        "b1f": b1f, "b1h": b1h, "b1v": b1v,
        "b2f": b2f,
        "b2h": b_ph[:, None].copy(),
        "b2v": b_pv[:, None].copy(),
    }


def kernel(**inputs) -> np.ndarray:
    x = np.asarray(inputs["x"]).astype(np.int32)
    assert x.shape == (B_FULL, NC_), x.shape

    shared = _prep_weights(inputs)
    nc = _get_nc()

    in_maps = []
    for core in range(NCORES):
        xs = x[core * BC:(core + 1) * BC]          # [BC, 25]
        xtc = np.zeros((128, BC), BF16NP)
        xtc[:NC_] = xs.T.astype(BF16NP)
        m = dict(shared)
        m["xt"] = xtc
        in_maps.append(m)

    res = run_bass_kernel_spmd(nc, in_maps, core_ids=list(range(NCORES)))
    global LAST_RESULTS
    LAST_RESULTS = res
    out = np.concatenate([res.results[i]["y"] for i in range(NCORES)], axis=0)
    return out


LAST_RESULTS = None
